# revision 28
# baseline (speedup 1.0000x reference)
"""FastWorkingMemory (DeltaNet-style recurrence with vector learning rate) on 8 TRN2 cores.

Reference computation (B=4, T=2048, D=1024, H=8, d=128):
    q = x @ Wq.T ; k = l2norm(x @ Wk.T) ; v = l2norm(x @ Wv.T)   (per-head d=128)
    lr = sigmoid(x @ Wlr.T + b_lr)
    scan over t:  v_old = S k_t ; S += (lr_t * (v_t - v_old)) k_t^T ; o_t = S q_t
    y = o @ Wo.T

Sharding: core c -> batch b = c//2, heads hg = c%2 (4 heads each). Each core computes a
partial y (its heads' contribution through Wo); host sums the two partials per batch.

Device algorithm: chunked delta rule, chunk C=128. Per (head, chunk):
    A = K K^T strict-lower, G = K Q^T masked s<=t  ([s,t] layouts)
    Vold = K @ P            (P = S^T state, [j,i])
    R = lr * (V - Vold)
    U = (I + D)^-1 R,  D(X) = lr o (A_strict X)  -- truncated Neumann/Horner:
        Z'_k = -lr o (A (R + Z'_{k-1})),  U = R + Z'_M
    O^T = P^T Q^T + U^T G   (one PSUM accumulation group)
    P  += K_rows^T U
    y_chunk = O @ Wo_cols

Schedule: projections for window w+1 are emitted as fine-grained "filler"
steps interleaved into the serial Neumann chain of window w's scan, keeping
the PE continuously busy (both hiding the chain latency and holding the PE
at its ramped clock). All IO is fp16; weight/x DMAs are batched one per
matrix / per window to amortize HWDGE issue overhead.
"""

import numpy as np

B, T, D, H = 4, 2048, 1024, 8
d = D // H
HPC = 4            # heads per core
DH = HPC * d       # 512: packed head width
C = 128            # scan chunk
W = 256            # projection window (t)
NWIN = T // W      # 8
NSUB = W // C      # 2 chunks per window
NJ = D // 128      # 8 contraction tiles
NEUMANN_ITERS = 10
EPS = 1e-12

_prog_cache = {}


def _build_program(debug=False):
    def ssl_h(h):
        return slice(h * 128, (h + 1) * 128)

    import concourse.mybir as mybir
    import concourse.tile as tile
    from concourse import bacc
    from concourse.masks import make_identity, make_upper_triangular

    f32 = mybir.dt.float32
    f16 = mybir.dt.float16
    Alu = mybir.AluOpType
    Act = mybir.ActivationFunctionType

    nc = bacc.Bacc("TRN2", target_bir_lowering=False, debug=False, num_devices=8)

    xT = nc.dram_tensor("xT", [D, T], f16, kind="ExternalInput").ap()
    WqT = nc.dram_tensor("WqT", [D, DH], f16, kind="ExternalInput").ap()
    WkT = nc.dram_tensor("WkT", [D, DH], f16, kind="ExternalInput").ap()
    WvT = nc.dram_tensor("WvT", [D, DH], f16, kind="ExternalInput").ap()
    WlT = nc.dram_tensor("WlT", [D, DH], f16, kind="ExternalInput").ap()
    blr = nc.dram_tensor("blr", [1, DH], f32, kind="ExternalInput").ap()
    WoT = nc.dram_tensor("WoT", [DH, D], f16, kind="ExternalInput").ap()
    y = nc.dram_tensor("y", [T, D], f16, kind="ExternalOutput").ap()

    with tile.TileContext(nc) as tc:
        with (
            tc.tile_pool(name="consts", bufs=1) as consts,
            tc.tile_pool(name="weights", bufs=1) as wpool,
            tc.tile_pool(name="state", bufs=1) as state,
            tc.tile_pool(name="xwin", bufs=2) as xwin,
            tc.tile_pool(name="rows", bufs=2) as rows,
            tc.tile_pool(name="twin", bufs=2) as twin,
            tc.tile_pool(name="chunk", bufs=2) as chk,
            tc.tile_pool(name="nscratch", bufs=2) as nsc,
            tc.tile_pool(name="ps_work", bufs=3, space="PSUM") as ps_work,
            tc.tile_pool(name="ps_neu", bufs=2, space="PSUM") as ps_neu,
            tc.tile_pool(name="ps_proj", bufs=2, space="PSUM") as ps_proj,
            tc.tile_pool(name="ps_tr", bufs=1, space="PSUM") as ps_tr,
        ):
            # ---- constants ----
            ident = consts.tile([128, 128], f32, tag="ident")
            make_identity(nc, ident)
            ident16 = consts.tile([128, 128], f16, tag="ident16")
            nc.gpsimd.tensor_copy(ident16[:], ident[:])
            maskA1 = consts.tile([128, 128], f32, tag="maskA1")  # 1 where s<t
            make_upper_triangular(nc, maskA1, val=1.0, diag=False)
            maskG1 = consts.tile([128, 128], f32, tag="maskG1")  # 1 where s<=t
            make_upper_triangular(nc, maskG1, val=1.0, diag=True)
            maskA = consts.tile([128, DH], f32, tag="maskA")
            maskG = consts.tile([128, DH], f32, tag="maskG")
            for h in range(HPC):
                nc.gpsimd.tensor_copy(maskA[:, h * 128:(h + 1) * 128], maskA1[:, :])
                nc.gpsimd.tensor_copy(maskG[:, h * 128:(h + 1) * 128], maskG1[:, :])
            ones_row = consts.tile([1, 128], f16, tag="ones_row")
            nc.vector.memset(ones_row[:], 1.0)
            blr_f32 = consts.tile([1, DH], f32, tag="blr_f32")
            blr_sb = consts.tile([1, DH], f16, tag="blr_sb")

            # ---- resident weights: one batched DMA per matrix ----
            # wk first (K proj runs first); the rest are issued after window
            # 0's x DMA so the critical path isn't stuck behind them.
            wkB = wpool.tile([128, NJ * DH], f16, tag="wkB")
            wqB = wpool.tile([128, NJ * DH], f16, tag="wqB")
            wvB = wpool.tile([128, NJ * DH], f16, tag="wvB")
            wlB = wpool.tile([128, NJ * DH], f16, tag="wlB")
            woB = wpool.tile([128, HPC * D], f16, tag="woB")
            wk = [wkB[:, j * DH:(j + 1) * DH] for j in range(NJ)]
            wq = [wqB[:, j * DH:(j + 1) * DH] for j in range(NJ)]
            wv = [wvB[:, j * DH:(j + 1) * DH] for j in range(NJ)]
            wl = [wlB[:, j * DH:(j + 1) * DH] for j in range(NJ)]
            wo = [woB[:, h * D:(h + 1) * D] for h in range(HPC)]

            def load_w(eng, big, WT, nj, width):
                eng.dma_start(
                    big[:].rearrange("p (j o) -> p j o", j=nj),
                    WT.rearrange("(j p) o -> p j o", j=nj, p=128))

            # quarter-granularity wk loads: the first K-proj matmuls start as
            # soon as the first two j-tiles land
            for q in range(4):
                nc.sync.dma_start(
                    wkB[:, q * 2 * DH:(q + 1) * 2 * DH].rearrange("p (j o) -> p j o", j=2),
                    WkT[q * 256:(q + 1) * 256, :].rearrange("(j p) o -> p j o", j=2, p=128))

            def late_loads():
                # window-0 xt is already first on the scalar queue; weights
                # follow in first-use order split across both HWDGE queues
                load_w(nc.scalar, wvB, WvT, NJ, DH)
                load_w(nc.sync, wqB, WqT, NJ, DH)
                load_w(nc.scalar, woB, WoT, HPC, D)
                load_w(nc.sync, wlB, WlT, NJ, DH)
                nc.scalar.dma_start(blr_f32[:], blr[:])
                nc.gpsimd.tensor_copy(blr_sb[:], blr_f32[:])

            # ---- state ----
            # P = S^T per head; two independent head-group streams (2 heads each)
            P2 = [state.tile([128, 256], f32, tag=f"P2_{u}", name=f"P2_{u}") for u in range(2)]
            P2h = [state.tile([128, 256], f16, tag=f"P2h_{u}", name=f"P2h_{u}") for u in range(2)]
            for u in range(2):
                nc.vector.memset(P2[u][:], 0.0)
                nc.vector.memset(P2h[u][:], 0.0)

            # ================= projection steps (filler-grained) =============
            # Steps are tagged with the scan point that needs them: key
            # 2*w + s means "must be emitted before scan(w) chunk s". Chunk 0
            # of a window only needs the s=0 tiles, so s=1 projection work
            # remains available as chain filler during chunk 0.
            def build_proj_steps(w):
                xtB = xwin.tile([128, NJ * W], f16, tag="xtB", name=f"xtB_{w}")
                xt = [xtB[:, j * W:(j + 1) * W] for j in range(NJ)]
                kr = [rows.tile([128, DH], f16, tag=f"kr{s}", name=f"kr{s}_{w}") for s in range(NSUB)]
                vr = [rows.tile([128, DH], f16, tag=f"vr{s}", name=f"vr{s}_{w}") for s in range(NSUB)]
                lr = [rows.tile([128, DH], f32, tag=f"lr{s}", name=f"lr{s}_{w}") for s in range(NSUB)]
                ln = [rows.tile([128, DH], f32, tag=f"ln{s}", name=f"ln{s}_{w}") for s in range(NSUB)]
                ktw = twin.tile([128, HPC * W], f16, tag="ktw", name=f"ktw_{w}")
                qtw = twin.tile([128, HPC * W], f16, tag="qtw", name=f"qtw_{w}")
                kt3 = ktw[:].rearrange("p (h t) -> p h t", h=HPC)
                qt3 = qtw[:].rearrange("p (h t) -> p h t", h=HPC)
                rawK = [nsc.tile([128, DH], f32, tag=f"rawK{s}", name=f"rawK{s}_{w}") for s in range(NSUB)]
                rawV = [nsc.tile([128, DH], f32, tag=f"rawV{s}", name=f"rawV{s}_{w}") for s in range(NSUB)]
                ss = [nsc.tile([128, 2 * HPC], f32, tag=f"ss{s}", name=f"ss{s}_{w}") for s in range(NSUB)]
                rcp = [nsc.tile([128, 2 * HPC], f32, tag=f"rcp{s}", name=f"rcp{s}_{w}") for s in range(NSUB)]

                steps = []
                box = {}

                def _x():
                    if w == 0:
                        for half in range(2):
                            jsl = slice(half * 4 * W, (half + 1) * 4 * W)
                            dsl = slice(half * 512, (half + 1) * 512)
                            nc.scalar.dma_start(
                                xtB[:, jsl].rearrange("p (j t) -> p j t", j=4),
                                xT[dsl, w * W:(w + 1) * W].rearrange(
                                    "(j p) t -> p j t", j=4, p=128))
                        late_loads()
                    else:
                        nc.sync.dma_start(
                            xtB[:].rearrange("p (j t) -> p j t", j=NJ),
                            xT[:, w * W:(w + 1) * W].rearrange(
                                "(j p) t -> p j t", j=NJ, p=128))
                steps.append((300, _x, 0))

                def proj_group(wts, s, key, raw, col):
                    tsl = slice(s * 128, (s + 1) * 128)
                    for j in range(NJ):
                        def mm(j=j, s=s, wts=wts, key=key, tsl=tsl):
                            if j == 0:
                                box[key + str(s)] = ps_proj.tile(
                                    [128, DH], f32, tag="proj",
                                    name=f"ps{key}{s}_{w}")
                            nc.tensor.matmul(
                                box[key + str(s)][:], xt[j][:, tsl], wts[j],
                                start=(j == 0), stop=(j == NJ - 1))
                        steps.append((213, mm, 0))

                    def drain(key=key, s=s, raw=raw, col=col):
                        ps = box[key + str(s)]
                        nc.scalar.copy(raw[s][:], ps[:])
                        sq = nsc.tile([128, DH], f32, tag="nsq", name=f"sq{key}{s}_{w}")
                        nc.gpsimd.tensor_tensor(sq[:], raw[s][:], raw[s][:], Alu.mult)
                        nc.vector.tensor_reduce(
                            ss[s][:, col:col + HPC],
                            sq[:].rearrange("p (h i) -> p h i", h=HPC),
                            axis=mybir.AxisListType.X, op=Alu.add)
                    steps.append((500, drain, 0))

                if w != 0:
                    for s in range(NSUB):
                        proj_group(wk, s, "K", rawK, 0)
                    for s in range(NSUB):
                        proj_group(wv, s, "V", rawV, HPC)

                for s in range(NSUB):
                    if w == 0:
                        proj_group(wk, s, "K", rawK, 0)
                        proj_group(wv, s, "V", rawV, HPC)

                    def norm2(s=s):
                        # rcp = 1/max(sqrt(ss), eps); V half negated (vr = -Vhat)
                        nc.scalar.activation(rcp[s][:], ss[s][:], Act.Sqrt)
                        nc.vector.tensor_scalar(
                            out=rcp[s][:], in0=rcp[s][:],
                            scalar1=EPS, scalar2=None, op0=Alu.max)
                        nc.vector.reciprocal(rcp[s][:], rcp[s][:])
                        nc.vector.tensor_scalar(
                            out=rcp[s][:, HPC:], in0=rcp[s][:, HPC:],
                            scalar1=-1.0, scalar2=None, op0=Alu.mult)
                    steps.append((400, norm2, 0 if s == 0 else s))

                    def scale(s=s, raw=None, out_rows=None, col=0):
                        for raw, out_rows, col in ((rawK, kr, 0), (rawV, vr, HPC)):
                            for h in range(HPC):
                                hsl = ssl_h(h)
                                nc.gpsimd.tensor_scalar(
                                    out=out_rows[s][:, hsl], in0=raw[s][:, hsl],
                                    scalar1=rcp[s][:, col + h:col + h + 1],
                                    scalar2=None, op0=Alu.mult)
                    steps.append((900, scale, s))

                    tsl = slice(s * 128, (s + 1) * 128)
                    for j in range(NJ):
                        def mm(j=j, s=s, tsl=tsl):
                            if j == 0:
                                box["Q" + str(s)] = ps_proj.tile(
                                    [128, DH], f32, tag="proj", name=f"psQ{s}_{w}")
                            nc.tensor.matmul(
                                box["Q" + str(s)][:], xt[j][:, tsl], wq[j],
                                start=(j == 0), stop=(j == NJ - 1))
                        steps.append((213, mm, s))

                    def qdrain(s=s):
                        qr = nsc.tile([128, DH], f16, tag="qr", name=f"qr{s}_{w}")
                        box["qr" + str(s)] = qr
                        nc.scalar.copy(qr[:], box["Q" + str(s)][:])
                    steps.append((400, qdrain, s))

                    def ktr(s=s):
                        for h in range(HPC):
                            eng = nc.sync if h % 2 == 0 else nc.scalar
                            eng.dma_start_transpose(
                                kt3[:, h, s * 128:(s + 1) * 128],
                                kr[s][:, ssl_h(h)])
                    steps.append((600, ktr, s))

                    def qtr(s=s):
                        for h in range(HPC):
                            eng = nc.sync if h % 2 == 0 else nc.scalar
                            eng.dma_start_transpose(
                                qt3[:, h, s * 128:(s + 1) * 128],
                                box["qr" + str(s)][:, ssl_h(h)])
                    steps.append((600, qtr, s))

                    # LR projection (+bias) -> sigmoid -> negate
                    for j in range(NJ):
                        def mm(j=j, s=s, tsl=tsl):
                            if j == 0:
                                box["L" + str(s)] = ps_proj.tile(
                                    [128, DH], f32, tag="proj", name=f"psL{s}_{w}")
                            nc.tensor.matmul(
                                box["L" + str(s)][:], xt[j][:, tsl], wl[j],
                                start=(j == 0), stop=False)
                        steps.append((213, mm, s))

                    def bias_sig(s=s):
                        nc.tensor.matmul(
                            box["L" + str(s)][:], ones_row[:], blr_sb[:],
                            start=False, stop=True)
                        nc.scalar.activation(lr[s][:], box["L" + str(s)][:], Act.Sigmoid)
                        nc.gpsimd.tensor_scalar(
                            out=ln[s][:], in0=lr[s][:], scalar1=-1.0, scalar2=None,
                            op0=Alu.mult)
                    steps.append((700, bias_sig, s))

                return (kr, vr, lr, ln, kt3, qt3), steps

            # ===================== scan =====================
            deferred_y = []

            def emit_scan(w, tiles, pop, flush):
                kr, vr, lr, ln, kt3, qt3 = tiles

                def emit_y(wy, s, Ot2):
                    t0 = wy * W + s * 128
                    last = (wy == NWIN - 1 and s == NSUB - 1)
                    y_sb = chk.tile([128, 1024], f16, tag="y_sb", name=f"ysb_{w}_{s}")
                    for ot in range(2):
                        osl = slice(ot * 512, (ot + 1) * 512)
                        psy = ps_work.tile([128, 512], f32, tag="work", name=f"psy{ot}_{w}_{s}")
                        for h in range(HPC):
                            u, j = divmod(h, 2)
                            hsl = slice(j * 128, (j + 1) * 128)
                            nc.tensor.matmul(
                                psy[:], Ot2[u][:, hsl], wo[h][:, osl],
                                start=(h == 0), stop=(h == HPC - 1))
                        nc.scalar.copy(y_sb[:, osl], psy[:])
                        if last:
                            nc.sync.dma_start(y[t0:t0 + 128, osl], y_sb[:, osl])
                    if not last:
                        nc.sync.dma_start(y[t0:t0 + 128, :], y_sb[:])

                for s in range(NSUB):
                    flush(2 * w + s)
                    csl = slice(s * 128, (s + 1) * 128)
                    STR = (slice(0, 256), slice(256, 512))
                    HH = ((0, 1), (2, 3))

                    A2, G2, Rb2, zb2, Ot2 = [], [], [], [], []
                    for u in range(2):
                        ssl = STR[u]
                        # A = K K^T strict-lower -> fp16
                        psA = ps_work.tile([128, 256], f32, tag="work", name=f"psA{u}_{w}_{s}")
                        for j, h in enumerate(HH[u]):
                            hsl = slice(j * 128, (j + 1) * 128)
                            nc.tensor.matmul(
                                psA[:, hsl], kt3[:, h, csl], kt3[:, h, csl],
                                start=True, stop=True)
                        A4 = chk.tile([128, 256], f16, tag=f"A4_{u}", name=f"A4_{u}_{w}_{s}")
                        nc.vector.tensor_tensor(A4[:], psA[:], maskA[:, ssl], Alu.mult)
                        A2.append(A4)

                        # G = K Q^T masked s<=t
                        psG = ps_work.tile([128, 256], f32, tag="work", name=f"psG{u}_{w}_{s}")
                        for j, h in enumerate(HH[u]):
                            hsl = slice(j * 128, (j + 1) * 128)
                            nc.tensor.matmul(
                                psG[:, hsl], kt3[:, h, csl], qt3[:, h, csl],
                                start=True, stop=True)
                        G4 = chk.tile([128, 256], f16, tag=f"G4_{u}", name=f"G4_{u}_{w}_{s}")
                        nc.vector.tensor_tensor(G4[:], psG[:], maskG[:, ssl], Alu.mult)
                        G2.append(G4)

                        # Vold = K @ P (rows), R = lr*(V - Vold) = ln*(Vold - V)
                        psVo = ps_work.tile([128, 256], f32, tag="work", name=f"psVo{u}_{w}_{s}")
                        nc.tensor.matmul(
                            psVo[:], ident16[:], vr[s][:, ssl],
                            start=True, stop=False)
                        for j, h in enumerate(HH[u]):
                            hsl = slice(j * 128, (j + 1) * 128)
                            nc.tensor.matmul(
                                psVo[:, hsl], kt3[:, h, csl], P2h[u][:, hsl],
                                start=False, stop=True)
                        Rb = chk.tile([128, 256], f16, tag=f"Rb_{u}", name=f"Rb_{u}_{w}_{s}")
                        nc.vector.tensor_tensor(Rb[:], ln[s][:, ssl], psVo[:], Alu.mult)
                        Rb2.append(Rb)
                        zb2.append(None)

                    # deferred y of the previous chunk fills the pre-chain gap
                    if deferred_y:
                        emit_y(*deferred_y.pop(0))
                    pop(POP_PRE)

                    # Neumann/Horner, streams interleaved per iteration:
                    # Z'_k = -lr o (A @ (R + Z'_{k-1}))
                    for it in range(NEUMANN_ITERS):
                        psN2 = []
                        for u in range(2):
                            psN = ps_neu.tile([128, 256], f32, tag="neu", name=f"psN{u}_{w}_{s}_{it}")
                            for j in range(2):
                                hsl = slice(j * 128, (j + 1) * 128)
                                nc.tensor.matmul(
                                    psN[:, hsl], A2[u][:, hsl], Rb2[u][:, hsl],
                                    start=True, stop=(zb2[u] is None))
                                if zb2[u] is not None:
                                    nc.tensor.matmul(
                                        psN[:, hsl], A2[u][:, hsl], zb2[u][:, hsl],
                                        start=False, stop=True)
                            psN2.append(psN)
                        for u in range(2):
                            zb_new = chk.tile([128, 256], f16, tag=f"zb_{u}", name=f"zb_{u}_{w}_{s}_{it}")
                            nc.vector.tensor_tensor(zb_new[:], ln[s][:, STR[u]], psN2[u][:], Alu.mult)
                            zb2[u] = zb_new
                        pop(POP_ITER)

                    # U = R + Z'_M is never materialized: psO and psP are
                    # linear in U, so R (as Rb) and Z'_M accumulate as separate
                    # matmuls into the same PSUM group.
                    for u in range(2):
                        # O^T = P^T Q^T + U^T G   [i, (h,t)]
                        psO = ps_work.tile([128, 256], f32, tag="work", name=f"psO{u}_{w}_{s}")
                        for j, h in enumerate(HH[u]):
                            hsl = slice(j * 128, (j + 1) * 128)
                            nc.tensor.matmul(
                                psO[:, hsl], P2h[u][:, hsl], qt3[:, h, csl],
                                start=True, stop=False)
                            nc.tensor.matmul(
                                psO[:, hsl], Rb2[u][:, hsl], G2[u][:, hsl],
                                start=False, stop=False)
                            nc.tensor.matmul(
                                psO[:, hsl], zb2[u][:, hsl], G2[u][:, hsl],
                                start=False, stop=True)
                        Ot = chk.tile([128, 256], f16, tag=f"Ot_{u}", name=f"Ot_{u}_{w}_{s}")
                        nc.scalar.copy(Ot[:], psO[:])
                        Ot2.append(Ot)

                        # P += K_rows^T U
                        psP = ps_work.tile([128, 256], f32, tag="work", name=f"psP{u}_{w}_{s}")
                        for j, h in enumerate(HH[u]):
                            hsl = slice(j * 128, (j + 1) * 128)
                            nc.tensor.matmul(
                                psP[:, hsl], kr[s][:, ssl_h(h)], Rb2[u][:, hsl],
                                start=True, stop=False)
                            nc.tensor.matmul(
                                psP[:, hsl], kr[s][:, ssl_h(h)], zb2[u][:, hsl],
                                start=False, stop=True)
                        nc.vector.tensor_tensor(P2h[u][:], P2h[u][:], psP[:], Alu.add)
                        pop(POP_TAIL)

                    deferred_y.append((w, s, Ot2))

                if w == NWIN - 1:
                    while deferred_y:
                        emit_y(*deferred_y.pop(0))

            # ===================== window loop =====================
            from collections import deque
            pending = deque()

            def pop(budget):
                while pending and budget > 0:
                    key, ns, fn = pending.popleft()
                    fn()
                    budget -= ns

            def flush(key):
                while pending and pending[0][0] <= key:
                    _, _, fn = pending.popleft()
                    fn()

            tiles_cur, steps0 = build_proj_steps(0)
            pending.extend((c, ns, fn) for ns, fn, c in steps0)
            for w in range(NWIN):
                if w + 1 < NWIN:
                    tiles_next, steps = build_proj_steps(w + 1)
                    pending.extend((2 * (w + 1) + c, ns, fn) for ns, fn, c in steps)
                else:
                    tiles_next = None
                emit_scan(w, tiles_cur, pop, flush)
                tiles_cur = tiles_next
            flush(10 ** 9)

    nc.compile()
    return nc


def get_program(debug=False):
    key = "nc_dbg" if debug else "nc"
    if key not in _prog_cache:
        _prog_cache[key] = _build_program(debug)
    return _prog_cache[key]


def kernel(x, Wq, Wk, Wv, Wo, Wlr, b_lr):
    from concourse import bass_utils

    nc = get_program()
    x = np.asarray(x, np.float16)
    Wq = np.asarray(Wq, np.float16)
    Wk = np.asarray(Wk, np.float16)
    Wv = np.asarray(Wv, np.float16)
    Wo = np.asarray(Wo, np.float16)
    Wlr = np.asarray(Wlr, np.float16)
    b_lr = np.asarray(b_lr, np.float32)

    in_maps = []
    for c in range(8):
        b, hg = divmod(c, 2)
        rs = slice(hg * DH, (hg + 1) * DH)   # head-sliced output rows of W*
        in_maps.append({
            "xT": np.ascontiguousarray(x[b].T),
            "WqT": np.ascontiguousarray(Wq[rs, :].T),
            "WkT": np.ascontiguousarray(Wk[rs, :].T),
            "WvT": np.ascontiguousarray(Wv[rs, :].T),
            "WlT": np.ascontiguousarray(Wlr[rs, :].T),
            "blr": np.ascontiguousarray(b_lr[rs][None, :]),
            "WoT": np.ascontiguousarray(Wo[:, rs].T),
        })
    res = bass_utils.run_bass_kernel_spmd(nc, in_maps, core_ids=list(range(8)))
    out = np.empty((B, T, D), np.float32)
    for b in range(B):
        out[b] = (res.results[2 * b]["y"].astype(np.float32)
                  + res.results[2 * b + 1]["y"].astype(np.float32))
    return out


# revision 29
# speedup vs baseline: 1.1210x; 1.1210x over previous
"""FastWorkingMemory (DeltaNet-style recurrence with vector learning rate) on 8 TRN2 cores.

Reference computation (B=4, T=2048, D=1024, H=8, d=128):
    q = x @ Wq.T ; k = l2norm(x @ Wk.T) ; v = l2norm(x @ Wv.T)   (per-head d=128)
    lr = sigmoid(x @ Wlr.T + b_lr)
    scan over t:  v_old = S k_t ; S += (lr_t * (v_t - v_old)) k_t^T ; o_t = S q_t
    y = o @ Wo.T

Sharding: core c -> batch b = c//2, heads hg = c%2 (4 heads each). Each core computes a
partial y (its heads' contribution through Wo); host sums the two partials per batch.

Device algorithm: chunked delta rule, chunk C=128. Per (head, chunk):
    A = K K^T strict-lower, G = K Q^T masked s<=t  ([s,t] layouts)
    Vold = K @ P            (P = S^T state, [j,i])
    R = lr * (V - Vold)
    U = (I + D)^-1 R,  D(X) = lr o (A_strict X)  -- truncated Neumann/Horner:
        Z'_k = -lr o (A (R + Z'_{k-1})),  U = R + Z'_M
    O^T = P^T Q^T + U^T G   (one PSUM accumulation group)
    P  += K_rows^T U
    y_chunk = O @ Wo_cols

Schedule: projections for window w+1 are emitted as fine-grained "filler"
steps interleaved into the serial Neumann chain of window w's scan, keeping
the PE continuously busy (both hiding the chain latency and holding the PE
at its ramped clock). All IO is fp16; weight/x DMAs are batched one per
matrix / per window to amortize HWDGE issue overhead.
"""

import numpy as np

B, T, D, H = 4, 2048, 1024, 8
d = D // H
HPC = 4            # heads per core
DH = HPC * d       # 512: packed head width
C = 128            # scan chunk
W = 256            # projection window (t)
NWIN = T // W      # 8
NSUB = W // C      # 2 chunks per window
NJ = D // 128      # 8 contraction tiles
NEUMANN_ITERS = 10
EPS = 1e-12

_prog_cache = {}


def _build_program(debug=False):
    def ssl_h(h):
        return slice(h * 128, (h + 1) * 128)

    import concourse.mybir as mybir
    import concourse.tile as tile
    from concourse import bacc
    from concourse.masks import make_identity, make_upper_triangular

    f32 = mybir.dt.float32
    f16 = mybir.dt.float16
    Alu = mybir.AluOpType
    Act = mybir.ActivationFunctionType

    nc = bacc.Bacc("TRN2", target_bir_lowering=False, debug=False, num_devices=8)

    xT = nc.dram_tensor("xT", [D, T], f16, kind="ExternalInput").ap()
    WqT = nc.dram_tensor("WqT", [D, DH], f16, kind="ExternalInput").ap()
    WkT = nc.dram_tensor("WkT", [D, DH], f16, kind="ExternalInput").ap()
    WvT = nc.dram_tensor("WvT", [D, DH], f16, kind="ExternalInput").ap()
    WlT = nc.dram_tensor("WlT", [D, DH], f16, kind="ExternalInput").ap()
    blr = nc.dram_tensor("blr", [1, DH], f32, kind="ExternalInput").ap()
    WoT = nc.dram_tensor("WoT", [DH, D], f16, kind="ExternalInput").ap()
    y = nc.dram_tensor("y", [T, D], f16, kind="ExternalOutput").ap()

    with tile.TileContext(nc) as tc:
        with (
            tc.tile_pool(name="consts", bufs=1) as consts,
            tc.tile_pool(name="weights", bufs=1) as wpool,
            tc.tile_pool(name="state", bufs=1) as state,
            tc.tile_pool(name="xwin", bufs=2) as xwin,
            tc.tile_pool(name="rows", bufs=2) as rows,
            tc.tile_pool(name="twin", bufs=2) as twin,
            tc.tile_pool(name="chunk", bufs=2) as chk,
            tc.tile_pool(name="nscratch", bufs=2) as nsc,
            tc.tile_pool(name="ps_work", bufs=3, space="PSUM") as ps_work,
            tc.tile_pool(name="ps_neu", bufs=2, space="PSUM") as ps_neu,
            tc.tile_pool(name="ps_proj", bufs=2, space="PSUM") as ps_proj,
            tc.tile_pool(name="ps_tr", bufs=1, space="PSUM") as ps_tr,
        ):
            # ---- constants ----
            ident = consts.tile([128, 128], f32, tag="ident")
            make_identity(nc, ident)
            ident16 = consts.tile([128, 128], f16, tag="ident16")
            nc.gpsimd.tensor_copy(ident16[:], ident[:])
            maskA1 = consts.tile([128, 128], f32, tag="maskA1")  # 1 where s<t
            make_upper_triangular(nc, maskA1, val=1.0, diag=False)
            maskG1 = consts.tile([128, 128], f32, tag="maskG1")  # 1 where s<=t
            make_upper_triangular(nc, maskG1, val=1.0, diag=True)
            maskA = consts.tile([128, DH], f32, tag="maskA")
            maskG = consts.tile([128, DH], f32, tag="maskG")
            for h in range(HPC):
                nc.gpsimd.tensor_copy(maskA[:, h * 128:(h + 1) * 128], maskA1[:, :])
                nc.gpsimd.tensor_copy(maskG[:, h * 128:(h + 1) * 128], maskG1[:, :])
            ones_row = consts.tile([1, 128], f16, tag="ones_row")
            nc.vector.memset(ones_row[:], 1.0)
            blr_f32 = consts.tile([1, DH], f32, tag="blr_f32")
            blr_sb = consts.tile([1, DH], f16, tag="blr_sb")

            # ---- resident weights: one batched DMA per matrix ----
            # wk first (K proj runs first); the rest are issued after window
            # 0's x DMA so the critical path isn't stuck behind them.
            wkB = wpool.tile([128, NJ * DH], f16, tag="wkB")
            wqB = wpool.tile([128, NJ * DH], f16, tag="wqB")
            wvB = wpool.tile([128, NJ * DH], f16, tag="wvB")
            wlB = wpool.tile([128, NJ * DH], f16, tag="wlB")
            woB = wpool.tile([128, HPC * D], f16, tag="woB")
            wk = [wkB[:, j * DH:(j + 1) * DH] for j in range(NJ)]
            wq = [wqB[:, j * DH:(j + 1) * DH] for j in range(NJ)]
            wv = [wvB[:, j * DH:(j + 1) * DH] for j in range(NJ)]
            wl = [wlB[:, j * DH:(j + 1) * DH] for j in range(NJ)]
            wo = [woB[:, h * D:(h + 1) * D] for h in range(HPC)]

            def load_w(eng, big, WT, nj, width):
                eng.dma_start(
                    big[:].rearrange("p (j o) -> p j o", j=nj),
                    WT.rearrange("(j p) o -> p j o", j=nj, p=128))

            # quarter-granularity wk loads: the first K-proj matmuls start as
            # soon as the first two j-tiles land
            for q in range(4):
                nc.sync.dma_start(
                    wkB[:, q * 2 * DH:(q + 1) * 2 * DH].rearrange("p (j o) -> p j o", j=2),
                    WkT[q * 256:(q + 1) * 256, :].rearrange("(j p) o -> p j o", j=2, p=128))

            def late_loads():
                # window-0 xt is already first on the scalar queue; weights
                # follow in first-use order split across both HWDGE queues
                load_w(nc.scalar, wvB, WvT, NJ, DH)
                load_w(nc.sync, wqB, WqT, NJ, DH)
                load_w(nc.scalar, woB, WoT, HPC, D)
                load_w(nc.sync, wlB, WlT, NJ, DH)
                nc.scalar.dma_start(blr_f32[:], blr[:])
                nc.gpsimd.tensor_copy(blr_sb[:], blr_f32[:])

            # ---- state ----
            # P = S^T per head; two independent head-group streams (2 heads each)
            P2 = [state.tile([128, 256], f32, tag=f"P2_{u}", name=f"P2_{u}") for u in range(2)]
            P2h = [state.tile([128, 256], f16, tag=f"P2h_{u}", name=f"P2h_{u}") for u in range(2)]
            for u in range(2):
                nc.vector.memset(P2[u][:], 0.0)
                nc.vector.memset(P2h[u][:], 0.0)

            # ================= projection steps (filler-grained) =============
            # Steps are tagged with the scan point that needs them: key
            # 2*w + s means "must be emitted before scan(w) chunk s". Chunk 0
            # of a window only needs the s=0 tiles, so s=1 projection work
            # remains available as chain filler during chunk 0.
            def build_proj_steps(w):
                xtB = xwin.tile([128, NJ * W], f16, tag="xtB", name=f"xtB_{w}")
                xt = [xtB[:, j * W:(j + 1) * W] for j in range(NJ)]
                kr = [rows.tile([128, DH], f16, tag=f"kr{s}", name=f"kr{s}_{w}") for s in range(NSUB)]
                vr = [rows.tile([128, DH], f16, tag=f"vr{s}", name=f"vr{s}_{w}") for s in range(NSUB)]
                lr = [rows.tile([128, DH], f32, tag=f"lr{s}", name=f"lr{s}_{w}") for s in range(NSUB)]
                ln = [rows.tile([128, DH], f32, tag=f"ln{s}", name=f"ln{s}_{w}") for s in range(NSUB)]
                ktw = twin.tile([128, HPC * W], f16, tag="ktw", name=f"ktw_{w}")
                qtw = twin.tile([128, HPC * W], f16, tag="qtw", name=f"qtw_{w}")
                kt3 = ktw[:].rearrange("p (h t) -> p h t", h=HPC)
                qt3 = qtw[:].rearrange("p (h t) -> p h t", h=HPC)
                rawK = [nsc.tile([128, DH], f32, tag=f"rawK{s}", name=f"rawK{s}_{w}") for s in range(NSUB)]
                rawV = [nsc.tile([128, DH], f32, tag=f"rawV{s}", name=f"rawV{s}_{w}") for s in range(NSUB)]
                ss = [nsc.tile([128, 2 * HPC], f32, tag=f"ss{s}", name=f"ss{s}_{w}") for s in range(NSUB)]
                rcp = [nsc.tile([128, 2 * HPC], f32, tag=f"rcp{s}", name=f"rcp{s}_{w}") for s in range(NSUB)]

                steps = []
                box = {}

                def _x():
                    if w == 0:
                        for half in range(2):
                            jsl = slice(half * 4 * W, (half + 1) * 4 * W)
                            dsl = slice(half * 512, (half + 1) * 512)
                            nc.scalar.dma_start(
                                xtB[:, jsl].rearrange("p (j t) -> p j t", j=4),
                                xT[dsl, w * W:(w + 1) * W].rearrange(
                                    "(j p) t -> p j t", j=4, p=128))
                        late_loads()
                    else:
                        nc.sync.dma_start(
                            xtB[:].rearrange("p (j t) -> p j t", j=NJ),
                            xT[:, w * W:(w + 1) * W].rearrange(
                                "(j p) t -> p j t", j=NJ, p=128))
                steps.append((300, _x, 0))

                def proj_group(wts, s, key, raw, col):
                    tsl = slice(s * 128, (s + 1) * 128)
                    for j in range(NJ):
                        def mm(j=j, s=s, wts=wts, key=key, tsl=tsl):
                            if j == 0:
                                box[key + str(s)] = ps_proj.tile(
                                    [128, DH], f32, tag="proj",
                                    name=f"ps{key}{s}_{w}")
                            nc.tensor.matmul(
                                box[key + str(s)][:], xt[j][:, tsl], wts[j],
                                start=(j == 0), stop=(j == NJ - 1))
                        steps.append((213, mm, 0))

                    def drain(key=key, s=s, raw=raw, col=col):
                        ps = box[key + str(s)]
                        nc.scalar.copy(raw[s][:], ps[:])
                        sq = nsc.tile([128, DH], f32, tag="nsq", name=f"sq{key}{s}_{w}")
                        nc.gpsimd.tensor_tensor(sq[:], raw[s][:], raw[s][:], Alu.mult)
                        nc.vector.tensor_reduce(
                            ss[s][:, col:col + HPC],
                            sq[:].rearrange("p (h i) -> p h i", h=HPC),
                            axis=mybir.AxisListType.X, op=Alu.add)
                    steps.append((500, drain, 0))

                if w != 0:
                    for s in range(NSUB):
                        proj_group(wk, s, "K", rawK, 0)
                    for s in range(NSUB):
                        proj_group(wv, s, "V", rawV, HPC)

                for s in range(NSUB):
                    if w == 0:
                        proj_group(wk, s, "K", rawK, 0)
                        proj_group(wv, s, "V", rawV, HPC)

                    def norm2(s=s):
                        # rcp = 1/max(sqrt(ss), eps); V half negated (vr = -Vhat)
                        nc.scalar.activation(rcp[s][:], ss[s][:], Act.Sqrt)
                        nc.vector.tensor_scalar(
                            out=rcp[s][:], in0=rcp[s][:],
                            scalar1=EPS, scalar2=None, op0=Alu.max)
                        nc.vector.reciprocal(rcp[s][:], rcp[s][:])
                        nc.vector.tensor_scalar(
                            out=rcp[s][:, HPC:], in0=rcp[s][:, HPC:],
                            scalar1=-1.0, scalar2=None, op0=Alu.mult)
                    steps.append((400, norm2, 0 if s == 0 else s))

                    def scale(s=s, raw=None, out_rows=None, col=0):
                        for raw, out_rows, col in ((rawK, kr, 0), (rawV, vr, HPC)):
                            for h in range(HPC):
                                hsl = ssl_h(h)
                                nc.gpsimd.tensor_scalar(
                                    out=out_rows[s][:, hsl], in0=raw[s][:, hsl],
                                    scalar1=rcp[s][:, col + h:col + h + 1],
                                    scalar2=None, op0=Alu.mult)
                    steps.append((900, scale, s))

                    tsl = slice(s * 128, (s + 1) * 128)
                    for j in range(NJ):
                        def mm(j=j, s=s, tsl=tsl):
                            if j == 0:
                                box["Q" + str(s)] = ps_proj.tile(
                                    [128, DH], f32, tag="proj", name=f"psQ{s}_{w}")
                            nc.tensor.matmul(
                                box["Q" + str(s)][:], xt[j][:, tsl], wq[j],
                                start=(j == 0), stop=(j == NJ - 1))
                        steps.append((213, mm, s))

                    def qdrain(s=s):
                        qr = nsc.tile([128, DH], f16, tag="qr", name=f"qr{s}_{w}")
                        box["qr" + str(s)] = qr
                        nc.scalar.copy(qr[:], box["Q" + str(s)][:])
                    steps.append((400, qdrain, s))

                    def ktr(s=s):
                        pst = ps_tr.tile([128, DH], f16, tag="tr", name=f"ptk{s}_{w}")
                        for h in range(HPC):
                            hsl = ssl_h(h)
                            nc.tensor.transpose(pst[:, hsl], kr[s][:, hsl], ident16[:])
                        nc.scalar.copy(
                            kt3[:, :, s * 128:(s + 1) * 128],
                            pst[:].rearrange("p (h t) -> p h t", h=HPC))
                    steps.append((600, ktr, s))

                    def qtr(s=s):
                        pst = ps_tr.tile([128, DH], f16, tag="tr", name=f"ptq{s}_{w}")
                        for h in range(HPC):
                            hsl = ssl_h(h)
                            nc.tensor.transpose(
                                pst[:, hsl], box["qr" + str(s)][:, hsl], ident16[:])
                        nc.scalar.copy(
                            qt3[:, :, s * 128:(s + 1) * 128],
                            pst[:].rearrange("p (h t) -> p h t", h=HPC))
                    steps.append((600, qtr, s))

                    # LR projection (+bias) -> sigmoid -> negate
                    for j in range(NJ):
                        def mm(j=j, s=s, tsl=tsl):
                            if j == 0:
                                box["L" + str(s)] = ps_proj.tile(
                                    [128, DH], f32, tag="proj", name=f"psL{s}_{w}")
                            nc.tensor.matmul(
                                box["L" + str(s)][:], xt[j][:, tsl], wl[j],
                                start=(j == 0), stop=False)
                        steps.append((213, mm, s))

                    def bias_sig(s=s):
                        nc.tensor.matmul(
                            box["L" + str(s)][:], ones_row[:], blr_sb[:],
                            start=False, stop=True)
                        nc.scalar.activation(lr[s][:], box["L" + str(s)][:], Act.Sigmoid)
                        nc.gpsimd.tensor_scalar(
                            out=ln[s][:], in0=lr[s][:], scalar1=-1.0, scalar2=None,
                            op0=Alu.mult)
                    steps.append((700, bias_sig, s))

                return (kr, vr, lr, ln, kt3, qt3), steps

            # ===================== scan =====================
            deferred_y = []

            def emit_scan(w, tiles, pop, flush):
                kr, vr, lr, ln, kt3, qt3 = tiles

                def emit_y(wy, s, Ot2):
                    t0 = wy * W + s * 128
                    last = (wy == NWIN - 1 and s == NSUB - 1)
                    y_sb = chk.tile([128, 1024], f16, tag="y_sb", name=f"ysb_{w}_{s}")
                    for ot in range(2):
                        osl = slice(ot * 512, (ot + 1) * 512)
                        psy = ps_work.tile([128, 512], f32, tag="work", name=f"psy{ot}_{w}_{s}")
                        for h in range(HPC):
                            u, j = divmod(h, 2)
                            hsl = slice(j * 128, (j + 1) * 128)
                            nc.tensor.matmul(
                                psy[:], Ot2[u][:, hsl], wo[h][:, osl],
                                start=(h == 0), stop=(h == HPC - 1))
                        nc.scalar.copy(y_sb[:, osl], psy[:])
                        if last:
                            nc.sync.dma_start(y[t0:t0 + 128, osl], y_sb[:, osl])
                    if not last:
                        nc.sync.dma_start(y[t0:t0 + 128, :], y_sb[:])

                for s in range(NSUB):
                    flush(2 * w + s)
                    csl = slice(s * 128, (s + 1) * 128)
                    STR = (slice(0, 256), slice(256, 512))
                    HH = ((0, 1), (2, 3))

                    A2, G2, Rb2, zb2, Ot2 = [], [], [], [], []
                    for u in range(2):
                        ssl = STR[u]
                        # A = K K^T strict-lower -> fp16
                        psA = ps_work.tile([128, 256], f32, tag="work", name=f"psA{u}_{w}_{s}")
                        for j, h in enumerate(HH[u]):
                            hsl = slice(j * 128, (j + 1) * 128)
                            nc.tensor.matmul(
                                psA[:, hsl], kt3[:, h, csl], kt3[:, h, csl],
                                start=True, stop=True)
                        A4 = chk.tile([128, 256], f16, tag=f"A4_{u}", name=f"A4_{u}_{w}_{s}")
                        nc.vector.tensor_tensor(A4[:], psA[:], maskA[:, ssl], Alu.mult)
                        A2.append(A4)

                        # G = K Q^T masked s<=t
                        psG = ps_work.tile([128, 256], f32, tag="work", name=f"psG{u}_{w}_{s}")
                        for j, h in enumerate(HH[u]):
                            hsl = slice(j * 128, (j + 1) * 128)
                            nc.tensor.matmul(
                                psG[:, hsl], kt3[:, h, csl], qt3[:, h, csl],
                                start=True, stop=True)
                        G4 = chk.tile([128, 256], f16, tag=f"G4_{u}", name=f"G4_{u}_{w}_{s}")
                        nc.vector.tensor_tensor(G4[:], psG[:], maskG[:, ssl], Alu.mult)
                        G2.append(G4)

                        # Vold = K @ P (rows), R = lr*(V - Vold) = ln*(Vold - V)
                        psVo = ps_work.tile([128, 256], f32, tag="work", name=f"psVo{u}_{w}_{s}")
                        nc.tensor.matmul(
                            psVo[:], ident16[:], vr[s][:, ssl],
                            start=True, stop=False)
                        for j, h in enumerate(HH[u]):
                            hsl = slice(j * 128, (j + 1) * 128)
                            nc.tensor.matmul(
                                psVo[:, hsl], kt3[:, h, csl], P2h[u][:, hsl],
                                start=False, stop=True)
                        Rb = chk.tile([128, 256], f16, tag=f"Rb_{u}", name=f"Rb_{u}_{w}_{s}")
                        nc.vector.tensor_tensor(Rb[:], ln[s][:, ssl], psVo[:], Alu.mult)
                        Rb2.append(Rb)
                        zb2.append(None)

                    # deferred y of the previous chunk fills the pre-chain gap
                    if deferred_y:
                        emit_y(*deferred_y.pop(0))
                    pop(POP_PRE)

                    # Neumann/Horner, streams interleaved per iteration:
                    # Z'_k = -lr o (A @ (R + Z'_{k-1}))
                    for it in range(NEUMANN_ITERS):
                        psN2 = []
                        for u in range(2):
                            psN = ps_neu.tile([128, 256], f32, tag="neu", name=f"psN{u}_{w}_{s}_{it}")
                            for j in range(2):
                                hsl = slice(j * 128, (j + 1) * 128)
                                nc.tensor.matmul(
                                    psN[:, hsl], A2[u][:, hsl], Rb2[u][:, hsl],
                                    start=True, stop=(zb2[u] is None))
                                if zb2[u] is not None:
                                    nc.tensor.matmul(
                                        psN[:, hsl], A2[u][:, hsl], zb2[u][:, hsl],
                                        start=False, stop=True)
                            psN2.append(psN)
                        for u in range(2):
                            zb_new = chk.tile([128, 256], f16, tag=f"zb_{u}", name=f"zb_{u}_{w}_{s}_{it}")
                            nc.vector.tensor_tensor(zb_new[:], ln[s][:, STR[u]], psN2[u][:], Alu.mult)
                            zb2[u] = zb_new
                        pop(POP_ITER)

                    # U = R + Z'_M is never materialized: psO and psP are
                    # linear in U, so R (as Rb) and Z'_M accumulate as separate
                    # matmuls into the same PSUM group.
                    for u in range(2):
                        # O^T = P^T Q^T + U^T G   [i, (h,t)]
                        psO = ps_work.tile([128, 256], f32, tag="work", name=f"psO{u}_{w}_{s}")
                        for j, h in enumerate(HH[u]):
                            hsl = slice(j * 128, (j + 1) * 128)
                            nc.tensor.matmul(
                                psO[:, hsl], P2h[u][:, hsl], qt3[:, h, csl],
                                start=True, stop=False)
                            nc.tensor.matmul(
                                psO[:, hsl], Rb2[u][:, hsl], G2[u][:, hsl],
                                start=False, stop=False)
                            nc.tensor.matmul(
                                psO[:, hsl], zb2[u][:, hsl], G2[u][:, hsl],
                                start=False, stop=True)
                        Ot = chk.tile([128, 256], f16, tag=f"Ot_{u}", name=f"Ot_{u}_{w}_{s}")
                        nc.scalar.copy(Ot[:], psO[:])
                        Ot2.append(Ot)

                        # P += K_rows^T U
                        psP = ps_work.tile([128, 256], f32, tag="work", name=f"psP{u}_{w}_{s}")
                        for j, h in enumerate(HH[u]):
                            hsl = slice(j * 128, (j + 1) * 128)
                            nc.tensor.matmul(
                                psP[:, hsl], kr[s][:, ssl_h(h)], Rb2[u][:, hsl],
                                start=True, stop=False)
                            nc.tensor.matmul(
                                psP[:, hsl], kr[s][:, ssl_h(h)], zb2[u][:, hsl],
                                start=False, stop=True)
                        nc.vector.tensor_tensor(P2h[u][:], P2h[u][:], psP[:], Alu.add)
                        pop(POP_TAIL)

                    deferred_y.append((w, s, Ot2))

                if w == NWIN - 1:
                    while deferred_y:
                        emit_y(*deferred_y.pop(0))

            # ===================== window loop =====================
            from collections import deque
            pending = deque()

            def pop(budget):
                while pending and budget > 0:
                    key, ns, fn = pending.popleft()
                    fn()
                    budget -= ns

            def flush(key):
                while pending and pending[0][0] <= key:
                    _, _, fn = pending.popleft()
                    fn()

            tiles_cur, steps0 = build_proj_steps(0)
            pending.extend((c, ns, fn) for ns, fn, c in steps0)
            for w in range(NWIN):
                if w + 1 < NWIN:
                    tiles_next, steps = build_proj_steps(w + 1)
                    pending.extend((2 * (w + 1) + c, ns, fn) for ns, fn, c in steps)
                else:
                    tiles_next = None
                emit_scan(w, tiles_cur, pop, flush)
                tiles_cur = tiles_next
            flush(10 ** 9)

    nc.compile()
    return nc


def get_program(debug=False):
    key = "nc_dbg" if debug else "nc"
    if key not in _prog_cache:
        _prog_cache[key] = _build_program(debug)
    return _prog_cache[key]


def kernel(x, Wq, Wk, Wv, Wo, Wlr, b_lr):
    from concourse import bass_utils

    nc = get_program()
    x = np.asarray(x, np.float16)
    Wq = np.asarray(Wq, np.float16)
    Wk = np.asarray(Wk, np.float16)
    Wv = np.asarray(Wv, np.float16)
    Wo = np.asarray(Wo, np.float16)
    Wlr = np.asarray(Wlr, np.float16)
    b_lr = np.asarray(b_lr, np.float32)

    in_maps = []
    for c in range(8):
        b, hg = divmod(c, 2)
        rs = slice(hg * DH, (hg + 1) * DH)   # head-sliced output rows of W*
        in_maps.append({
            "xT": np.ascontiguousarray(x[b].T),
            "WqT": np.ascontiguousarray(Wq[rs, :].T),
            "WkT": np.ascontiguousarray(Wk[rs, :].T),
            "WvT": np.ascontiguousarray(Wv[rs, :].T),
            "WlT": np.ascontiguousarray(Wlr[rs, :].T),
            "blr": np.ascontiguousarray(b_lr[rs][None, :]),
            "WoT": np.ascontiguousarray(Wo[:, rs].T),
        })
    res = bass_utils.run_bass_kernel_spmd(nc, in_maps, core_ids=list(range(8)))
    out = np.empty((B, T, D), np.float32)
    for b in range(B):
        out[b] = (res.results[2 * b]["y"].astype(np.float32)
                  + res.results[2 * b + 1]["y"].astype(np.float32))
    return out


# revision 30
# speedup vs baseline: 1.1221x; 1.0011x over previous
"""FastWorkingMemory (DeltaNet-style recurrence with vector learning rate) on 8 TRN2 cores.

Reference computation (B=4, T=2048, D=1024, H=8, d=128):
    q = x @ Wq.T ; k = l2norm(x @ Wk.T) ; v = l2norm(x @ Wv.T)   (per-head d=128)
    lr = sigmoid(x @ Wlr.T + b_lr)
    scan over t:  v_old = S k_t ; S += (lr_t * (v_t - v_old)) k_t^T ; o_t = S q_t
    y = o @ Wo.T

Sharding: core c -> batch b = c//2, heads hg = c%2 (4 heads each). Each core computes a
partial y (its heads' contribution through Wo); host sums the two partials per batch.

Device algorithm: chunked delta rule, chunk C=128. Per (head, chunk):
    A = K K^T strict-lower, G = K Q^T masked s<=t  ([s,t] layouts)
    Vold = K @ P            (P = S^T state, [j,i])
    R = lr * (V - Vold)
    U = (I + D)^-1 R,  D(X) = lr o (A_strict X)  -- truncated Neumann/Horner:
        Z'_k = -lr o (A (R + Z'_{k-1})),  U = R + Z'_M
    O^T = P^T Q^T + U^T G   (one PSUM accumulation group)
    P  += K_rows^T U
    y_chunk = O @ Wo_cols

Schedule: projections for window w+1 are emitted as fine-grained "filler"
steps interleaved into the serial Neumann chain of window w's scan, keeping
the PE continuously busy (both hiding the chain latency and holding the PE
at its ramped clock). All IO is fp16; weight/x DMAs are batched one per
matrix / per window to amortize HWDGE issue overhead.
"""

import numpy as np

B, T, D, H = 4, 2048, 1024, 8
d = D // H
HPC = 4            # heads per core
DH = HPC * d       # 512: packed head width
C = 128            # scan chunk
W = 256            # projection window (t)
NWIN = T // W      # 8
NSUB = W // C      # 2 chunks per window
NJ = D // 128      # 8 contraction tiles
NEUMANN_ITERS = 10
EPS = 1e-12

_prog_cache = {}


def _build_program(debug=False):
    def ssl_h(h):
        return slice(h * 128, (h + 1) * 128)

    import concourse.mybir as mybir
    import concourse.tile as tile
    from concourse import bacc
    from concourse.masks import make_identity, make_upper_triangular

    f32 = mybir.dt.float32
    f16 = mybir.dt.float16
    Alu = mybir.AluOpType
    Act = mybir.ActivationFunctionType

    nc = bacc.Bacc("TRN2", target_bir_lowering=False, debug=False, num_devices=8)

    xT = nc.dram_tensor("xT", [D, T], f16, kind="ExternalInput").ap()
    WqT = nc.dram_tensor("WqT", [D, DH], f16, kind="ExternalInput").ap()
    WkT = nc.dram_tensor("WkT", [D, DH], f16, kind="ExternalInput").ap()
    WvT = nc.dram_tensor("WvT", [D, DH], f16, kind="ExternalInput").ap()
    WlT = nc.dram_tensor("WlT", [D, DH], f16, kind="ExternalInput").ap()
    blr = nc.dram_tensor("blr", [1, DH], f32, kind="ExternalInput").ap()
    WoT = nc.dram_tensor("WoT", [DH, D], f16, kind="ExternalInput").ap()
    y = nc.dram_tensor("y", [T, D], f16, kind="ExternalOutput").ap()

    with tile.TileContext(nc) as tc:
        with (
            tc.tile_pool(name="consts", bufs=1) as consts,
            tc.tile_pool(name="weights", bufs=1) as wpool,
            tc.tile_pool(name="state", bufs=1) as state,
            tc.tile_pool(name="xwin", bufs=2) as xwin,
            tc.tile_pool(name="rows", bufs=2) as rows,
            tc.tile_pool(name="twin", bufs=2) as twin,
            tc.tile_pool(name="chunk", bufs=2) as chk,
            tc.tile_pool(name="nscratch", bufs=2) as nsc,
            tc.tile_pool(name="ps_work", bufs=3, space="PSUM") as ps_work,
            tc.tile_pool(name="ps_neu", bufs=2, space="PSUM") as ps_neu,
            tc.tile_pool(name="ps_proj", bufs=2, space="PSUM") as ps_proj,
            tc.tile_pool(name="ps_tr", bufs=1, space="PSUM") as ps_tr,
        ):
            # ---- constants ----
            ident = consts.tile([128, 128], f32, tag="ident")
            make_identity(nc, ident)
            ident16 = consts.tile([128, 128], f16, tag="ident16")
            nc.gpsimd.tensor_copy(ident16[:], ident[:])
            maskA1 = consts.tile([128, 128], f32, tag="maskA1")  # 1 where s<t
            make_upper_triangular(nc, maskA1, val=1.0, diag=False)
            maskG1 = consts.tile([128, 128], f32, tag="maskG1")  # 1 where s<=t
            make_upper_triangular(nc, maskG1, val=1.0, diag=True)
            maskA = consts.tile([128, DH], f32, tag="maskA")
            maskG = consts.tile([128, DH], f32, tag="maskG")
            for h in range(HPC):
                nc.gpsimd.tensor_copy(maskA[:, h * 128:(h + 1) * 128], maskA1[:, :])
                nc.gpsimd.tensor_copy(maskG[:, h * 128:(h + 1) * 128], maskG1[:, :])
            ones_row = consts.tile([1, 128], f16, tag="ones_row")
            nc.vector.memset(ones_row[:], 1.0)
            blr_f32 = consts.tile([1, DH], f32, tag="blr_f32")
            blr_sb = consts.tile([1, DH], f16, tag="blr_sb")

            # ---- resident weights: one batched DMA per matrix ----
            # wk first (K proj runs first); the rest are issued after window
            # 0's x DMA so the critical path isn't stuck behind them.
            wkB = wpool.tile([128, NJ * DH], f16, tag="wkB")
            wqB = wpool.tile([128, NJ * DH], f16, tag="wqB")
            wvB = wpool.tile([128, NJ * DH], f16, tag="wvB")
            wlB = wpool.tile([128, NJ * DH], f16, tag="wlB")
            woB = wpool.tile([128, HPC * D], f16, tag="woB")
            wk = [wkB[:, j * DH:(j + 1) * DH] for j in range(NJ)]
            wq = [wqB[:, j * DH:(j + 1) * DH] for j in range(NJ)]
            wv = [wvB[:, j * DH:(j + 1) * DH] for j in range(NJ)]
            wl = [wlB[:, j * DH:(j + 1) * DH] for j in range(NJ)]
            wo = [woB[:, h * D:(h + 1) * D] for h in range(HPC)]

            def load_w(eng, big, WT, nj, width):
                eng.dma_start(
                    big[:].rearrange("p (j o) -> p j o", j=nj),
                    WT.rearrange("(j p) o -> p j o", j=nj, p=128))

            # quarter-granularity wk loads: the first K-proj matmuls start as
            # soon as the first two j-tiles land
            for q in range(4):
                nc.sync.dma_start(
                    wkB[:, q * 2 * DH:(q + 1) * 2 * DH].rearrange("p (j o) -> p j o", j=2),
                    WkT[q * 256:(q + 1) * 256, :].rearrange("(j p) o -> p j o", j=2, p=128))

            def late_loads():
                # window-0 xt is already first on the scalar queue; weights
                # follow in first-use order split across both HWDGE queues
                load_w(nc.scalar, wvB, WvT, NJ, DH)
                load_w(nc.sync, wqB, WqT, NJ, DH)
                load_w(nc.scalar, woB, WoT, HPC, D)
                load_w(nc.sync, wlB, WlT, NJ, DH)
                nc.scalar.dma_start(blr_f32[:], blr[:])
                nc.gpsimd.tensor_copy(blr_sb[:], blr_f32[:])

            # ---- state ----
            # P = S^T per head; two independent head-group streams (2 heads each)
            P2 = [state.tile([128, 256], f32, tag=f"P2_{u}", name=f"P2_{u}") for u in range(2)]
            P2h = [state.tile([128, 256], f16, tag=f"P2h_{u}", name=f"P2h_{u}") for u in range(2)]
            for u in range(2):
                nc.vector.memset(P2[u][:], 0.0)
                nc.vector.memset(P2h[u][:], 0.0)

            # ================= projection steps (filler-grained) =============
            # Steps are tagged with the scan point that needs them: key
            # 2*w + s means "must be emitted before scan(w) chunk s". Chunk 0
            # of a window only needs the s=0 tiles, so s=1 projection work
            # remains available as chain filler during chunk 0.
            def build_proj_steps(w):
                xtB = xwin.tile([128, NJ * W], f16, tag="xtB", name=f"xtB_{w}")
                xt = [xtB[:, j * W:(j + 1) * W] for j in range(NJ)]
                kr = [rows.tile([128, DH], f16, tag=f"kr{s}", name=f"kr{s}_{w}") for s in range(NSUB)]
                vr = [rows.tile([128, DH], f16, tag=f"vr{s}", name=f"vr{s}_{w}") for s in range(NSUB)]
                lr = [rows.tile([128, DH], f32, tag=f"lr{s}", name=f"lr{s}_{w}") for s in range(NSUB)]
                ln = [rows.tile([128, DH], f32, tag=f"ln{s}", name=f"ln{s}_{w}") for s in range(NSUB)]
                ktw = twin.tile([128, HPC * W], f16, tag="ktw", name=f"ktw_{w}")
                qtw = twin.tile([128, HPC * W], f16, tag="qtw", name=f"qtw_{w}")
                kt3 = ktw[:].rearrange("p (h t) -> p h t", h=HPC)
                qt3 = qtw[:].rearrange("p (h t) -> p h t", h=HPC)
                rawK = [nsc.tile([128, DH], f32, tag=f"rawK{s}", name=f"rawK{s}_{w}") for s in range(NSUB)]
                rawV = [nsc.tile([128, DH], f32, tag=f"rawV{s}", name=f"rawV{s}_{w}") for s in range(NSUB)]
                ss = [nsc.tile([128, 2 * HPC], f32, tag=f"ss{s}", name=f"ss{s}_{w}") for s in range(NSUB)]
                rcp = [nsc.tile([128, 2 * HPC], f32, tag=f"rcp{s}", name=f"rcp{s}_{w}") for s in range(NSUB)]

                steps = []
                box = {}

                def _x():
                    if w == 0:
                        for half in range(2):
                            jsl = slice(half * 4 * W, (half + 1) * 4 * W)
                            dsl = slice(half * 512, (half + 1) * 512)
                            nc.scalar.dma_start(
                                xtB[:, jsl].rearrange("p (j t) -> p j t", j=4),
                                xT[dsl, w * W:(w + 1) * W].rearrange(
                                    "(j p) t -> p j t", j=4, p=128))
                        late_loads()
                    else:
                        nc.sync.dma_start(
                            xtB[:].rearrange("p (j t) -> p j t", j=NJ),
                            xT[:, w * W:(w + 1) * W].rearrange(
                                "(j p) t -> p j t", j=NJ, p=128))
                steps.append((300, _x, 0))

                def proj_group(wts, s, key, raw, col):
                    tsl = slice(s * 128, (s + 1) * 128)
                    for j in range(NJ):
                        def mm(j=j, s=s, wts=wts, key=key, tsl=tsl):
                            if j == 0:
                                box[key + str(s)] = ps_proj.tile(
                                    [128, DH], f32, tag="proj",
                                    name=f"ps{key}{s}_{w}")
                            nc.tensor.matmul(
                                box[key + str(s)][:], xt[j][:, tsl], wts[j],
                                start=(j == 0), stop=(j == NJ - 1))
                        steps.append((213, mm, 0))

                    def drain(key=key, s=s, raw=raw, col=col):
                        ps = box[key + str(s)]
                        nc.scalar.copy(raw[s][:], ps[:])
                        sq = nsc.tile([128, DH], f32, tag="nsq", name=f"sq{key}{s}_{w}")
                        nc.gpsimd.tensor_tensor(sq[:], raw[s][:], raw[s][:], Alu.mult)
                        nc.vector.tensor_reduce(
                            ss[s][:, col:col + HPC],
                            sq[:].rearrange("p (h i) -> p h i", h=HPC),
                            axis=mybir.AxisListType.X, op=Alu.add)
                    steps.append((500, drain, 0))

                if w != 0:
                    for s in range(NSUB):
                        proj_group(wk, s, "K", rawK, 0)
                    for s in range(NSUB):
                        proj_group(wv, s, "V", rawV, HPC)

                for s in range(NSUB):
                    if w == 0:
                        proj_group(wk, s, "K", rawK, 0)
                        proj_group(wv, s, "V", rawV, HPC)

                    def norm2(s=s):
                        # rcp = 1/max(sqrt(ss), eps); V half negated (vr = -Vhat)
                        nc.scalar.activation(rcp[s][:], ss[s][:], Act.Sqrt)
                        nc.vector.tensor_scalar(
                            out=rcp[s][:], in0=rcp[s][:],
                            scalar1=EPS, scalar2=None, op0=Alu.max)
                        nc.vector.reciprocal(rcp[s][:], rcp[s][:])
                        nc.vector.tensor_scalar(
                            out=rcp[s][:, HPC:], in0=rcp[s][:, HPC:],
                            scalar1=-1.0, scalar2=None, op0=Alu.mult)
                    steps.append((400, norm2, 0 if s == 0 else s))

                    def scale(s=s, raw=None, out_rows=None, col=0):
                        for raw, out_rows, col in ((rawK, kr, 0), (rawV, vr, HPC)):
                            for h in range(HPC):
                                hsl = ssl_h(h)
                                nc.gpsimd.tensor_scalar(
                                    out=out_rows[s][:, hsl], in0=raw[s][:, hsl],
                                    scalar1=rcp[s][:, col + h:col + h + 1],
                                    scalar2=None, op0=Alu.mult)
                    steps.append((900, scale, s))

                    tsl = slice(s * 128, (s + 1) * 128)
                    for j in range(NJ):
                        def mm(j=j, s=s, tsl=tsl):
                            if j == 0:
                                box["Q" + str(s)] = ps_proj.tile(
                                    [128, DH], f32, tag="proj", name=f"psQ{s}_{w}")
                            nc.tensor.matmul(
                                box["Q" + str(s)][:], xt[j][:, tsl], wq[j],
                                start=(j == 0), stop=(j == NJ - 1))
                        steps.append((213, mm, s))

                    def qdrain(s=s):
                        qr = nsc.tile([128, DH], f16, tag="qr", name=f"qr{s}_{w}")
                        box["qr" + str(s)] = qr
                        nc.scalar.copy(qr[:], box["Q" + str(s)][:])
                    steps.append((400, qdrain, s))

                    def ktr(s=s):
                        pst = ps_tr.tile([128, DH], f16, tag="tr", name=f"ptk{s}_{w}")
                        for h in range(HPC):
                            hsl = ssl_h(h)
                            nc.tensor.transpose(pst[:, hsl], kr[s][:, hsl], ident16[:])
                        nc.scalar.copy(
                            kt3[:, :, s * 128:(s + 1) * 128],
                            pst[:].rearrange("p (h t) -> p h t", h=HPC))
                    steps.append((600, ktr, s))

                    def qtr(s=s):
                        pst = ps_tr.tile([128, DH], f16, tag="tr", name=f"ptq{s}_{w}")
                        for h in range(HPC):
                            hsl = ssl_h(h)
                            nc.tensor.transpose(
                                pst[:, hsl], box["qr" + str(s)][:, hsl], ident16[:])
                        nc.scalar.copy(
                            qt3[:, :, s * 128:(s + 1) * 128],
                            pst[:].rearrange("p (h t) -> p h t", h=HPC))
                    steps.append((600, qtr, s))

                    # LR projection (+bias) -> sigmoid -> negate
                    for j in range(NJ):
                        def mm(j=j, s=s, tsl=tsl):
                            if j == 0:
                                box["L" + str(s)] = ps_proj.tile(
                                    [128, DH], f32, tag="proj", name=f"psL{s}_{w}")
                            nc.tensor.matmul(
                                box["L" + str(s)][:], xt[j][:, tsl], wl[j],
                                start=(j == 0), stop=False)
                        steps.append((213, mm, s))

                    def bias_sig(s=s):
                        nc.tensor.matmul(
                            box["L" + str(s)][:], ones_row[:], blr_sb[:],
                            start=False, stop=True)
                        nc.scalar.activation(lr[s][:], box["L" + str(s)][:], Act.Sigmoid)
                        nc.gpsimd.tensor_scalar(
                            out=ln[s][:], in0=lr[s][:], scalar1=-1.0, scalar2=None,
                            op0=Alu.mult)
                    steps.append((700, bias_sig, s))

                return (kr, vr, lr, ln, kt3, qt3), steps

            # ===================== scan =====================
            deferred_y = []

            def emit_scan(w, tiles, pop, flush):
                kr, vr, lr, ln, kt3, qt3 = tiles

                def emit_y(wy, s, Ot2):
                    t0 = wy * W + s * 128
                    last = (wy == NWIN - 1 and s == NSUB - 1)
                    y_sb = chk.tile([128, 1024], f16, tag="y_sb", name=f"ysb_{w}_{s}")
                    for ot in range(2):
                        osl = slice(ot * 512, (ot + 1) * 512)
                        psy = ps_work.tile([128, 512], f32, tag="work", name=f"psy{ot}_{w}_{s}")
                        for h in range(HPC):
                            u, j = divmod(h, 2)
                            hsl = slice(j * 128, (j + 1) * 128)
                            nc.tensor.matmul(
                                psy[:], Ot2[u][:, hsl], wo[h][:, osl],
                                start=(h == 0), stop=(h == HPC - 1))
                        nc.scalar.copy(y_sb[:, osl], psy[:])
                        if last:
                            nc.sync.dma_start(y[t0:t0 + 128, osl], y_sb[:, osl])
                    if not last:
                        nc.sync.dma_start(y[t0:t0 + 128, :], y_sb[:])

                for s in range(NSUB):
                    flush(2 * w + s)
                    csl = slice(s * 128, (s + 1) * 128)
                    STR = (slice(0, 256), slice(256, 512))
                    HH = ((0, 1), (2, 3))

                    A2, G2, Rb2, zb2, Ot2 = [], [], [], [], []
                    # Vold and A first (they gate the Neumann chain); the G
                    # matrices are only needed at the end of the chunk.
                    for u in range(2):
                        ssl = STR[u]
                        # Vold = K @ P (rows), R = lr*(V - Vold) = ln*(Vold - V)
                        psVo = ps_work.tile([128, 256], f32, tag="work", name=f"psVo{u}_{w}_{s}")
                        nc.tensor.matmul(
                            psVo[:], ident16[:], vr[s][:, ssl],
                            start=True, stop=False)
                        for j, h in enumerate(HH[u]):
                            hsl = slice(j * 128, (j + 1) * 128)
                            nc.tensor.matmul(
                                psVo[:, hsl], kt3[:, h, csl], P2h[u][:, hsl],
                                start=False, stop=True)
                        Rb = chk.tile([128, 256], f16, tag=f"Rb_{u}", name=f"Rb_{u}_{w}_{s}")
                        nc.vector.tensor_tensor(Rb[:], ln[s][:, ssl], psVo[:], Alu.mult)
                        Rb2.append(Rb)
                        zb2.append(None)

                        # A = K K^T strict-lower -> fp16
                        psA = ps_work.tile([128, 256], f32, tag="work", name=f"psA{u}_{w}_{s}")
                        for j, h in enumerate(HH[u]):
                            hsl = slice(j * 128, (j + 1) * 128)
                            nc.tensor.matmul(
                                psA[:, hsl], kt3[:, h, csl], kt3[:, h, csl],
                                start=True, stop=True)
                        A4 = chk.tile([128, 256], f16, tag=f"A4_{u}", name=f"A4_{u}_{w}_{s}")
                        nc.vector.tensor_tensor(A4[:], psA[:], maskA[:, ssl], Alu.mult)
                        A2.append(A4)

                    for u in range(2):
                        ssl = STR[u]
                        # G = K Q^T masked s<=t
                        psG = ps_work.tile([128, 256], f32, tag="work", name=f"psG{u}_{w}_{s}")
                        for j, h in enumerate(HH[u]):
                            hsl = slice(j * 128, (j + 1) * 128)
                            nc.tensor.matmul(
                                psG[:, hsl], kt3[:, h, csl], qt3[:, h, csl],
                                start=True, stop=True)
                        G4 = chk.tile([128, 256], f16, tag=f"G4_{u}", name=f"G4_{u}_{w}_{s}")
                        nc.vector.tensor_tensor(G4[:], psG[:], maskG[:, ssl], Alu.mult)
                        G2.append(G4)

                    # deferred y of the previous chunk fills the pre-chain gap
                    if deferred_y:
                        emit_y(*deferred_y.pop(0))
                    pop(POP_PRE)

                    # Neumann/Horner, streams interleaved per iteration:
                    # Z'_k = -lr o (A @ (R + Z'_{k-1}))
                    for it in range(NEUMANN_ITERS):
                        psN2 = []
                        for u in range(2):
                            psN = ps_neu.tile([128, 256], f32, tag="neu", name=f"psN{u}_{w}_{s}_{it}")
                            for j in range(2):
                                hsl = slice(j * 128, (j + 1) * 128)
                                nc.tensor.matmul(
                                    psN[:, hsl], A2[u][:, hsl], Rb2[u][:, hsl],
                                    start=True, stop=(zb2[u] is None))
                                if zb2[u] is not None:
                                    nc.tensor.matmul(
                                        psN[:, hsl], A2[u][:, hsl], zb2[u][:, hsl],
                                        start=False, stop=True)
                            psN2.append(psN)
                        for u in range(2):
                            zb_new = chk.tile([128, 256], f16, tag=f"zb_{u}", name=f"zb_{u}_{w}_{s}_{it}")
                            nc.vector.tensor_tensor(zb_new[:], ln[s][:, STR[u]], psN2[u][:], Alu.mult)
                            zb2[u] = zb_new
                        pop(POP_ITER)

                    # U = R + Z'_M is never materialized: psO and psP are
                    # linear in U, so R (as Rb) and Z'_M accumulate as separate
                    # matmuls into the same PSUM group.
                    for u in range(2):
                        # O^T = P^T Q^T + U^T G   [i, (h,t)]
                        psO = ps_work.tile([128, 256], f32, tag="work", name=f"psO{u}_{w}_{s}")
                        for j, h in enumerate(HH[u]):
                            hsl = slice(j * 128, (j + 1) * 128)
                            nc.tensor.matmul(
                                psO[:, hsl], P2h[u][:, hsl], qt3[:, h, csl],
                                start=True, stop=False)
                            nc.tensor.matmul(
                                psO[:, hsl], Rb2[u][:, hsl], G2[u][:, hsl],
                                start=False, stop=False)
                            nc.tensor.matmul(
                                psO[:, hsl], zb2[u][:, hsl], G2[u][:, hsl],
                                start=False, stop=True)
                        Ot = chk.tile([128, 256], f16, tag=f"Ot_{u}", name=f"Ot_{u}_{w}_{s}")
                        nc.scalar.copy(Ot[:], psO[:])
                        Ot2.append(Ot)

                        # P += K_rows^T U
                        psP = ps_work.tile([128, 256], f32, tag="work", name=f"psP{u}_{w}_{s}")
                        for j, h in enumerate(HH[u]):
                            hsl = slice(j * 128, (j + 1) * 128)
                            nc.tensor.matmul(
                                psP[:, hsl], kr[s][:, ssl_h(h)], Rb2[u][:, hsl],
                                start=True, stop=False)
                            nc.tensor.matmul(
                                psP[:, hsl], kr[s][:, ssl_h(h)], zb2[u][:, hsl],
                                start=False, stop=True)
                        nc.vector.tensor_tensor(P2h[u][:], P2h[u][:], psP[:], Alu.add)
                        pop(POP_TAIL)

                    deferred_y.append((w, s, Ot2))

                if w == NWIN - 1:
                    while deferred_y:
                        emit_y(*deferred_y.pop(0))

            # ===================== window loop =====================
            from collections import deque
            pending = deque()

            def pop(budget):
                while pending and budget > 0:
                    key, ns, fn = pending.popleft()
                    fn()
                    budget -= ns

            def flush(key):
                while pending and pending[0][0] <= key:
                    _, _, fn = pending.popleft()
                    fn()

            tiles_cur, steps0 = build_proj_steps(0)
            pending.extend((c, ns, fn) for ns, fn, c in steps0)
            for w in range(NWIN):
                if w + 1 < NWIN:
                    tiles_next, steps = build_proj_steps(w + 1)
                    pending.extend((2 * (w + 1) + c, ns, fn) for ns, fn, c in steps)
                else:
                    tiles_next = None
                emit_scan(w, tiles_cur, pop, flush)
                tiles_cur = tiles_next
            flush(10 ** 9)

    nc.compile()
    return nc


def get_program(debug=False):
    key = "nc_dbg" if debug else "nc"
    if key not in _prog_cache:
        _prog_cache[key] = _build_program(debug)
    return _prog_cache[key]


def kernel(x, Wq, Wk, Wv, Wo, Wlr, b_lr):
    from concourse import bass_utils

    nc = get_program()
    x = np.asarray(x, np.float16)
    Wq = np.asarray(Wq, np.float16)
    Wk = np.asarray(Wk, np.float16)
    Wv = np.asarray(Wv, np.float16)
    Wo = np.asarray(Wo, np.float16)
    Wlr = np.asarray(Wlr, np.float16)
    b_lr = np.asarray(b_lr, np.float32)

    in_maps = []
    for c in range(8):
        b, hg = divmod(c, 2)
        rs = slice(hg * DH, (hg + 1) * DH)   # head-sliced output rows of W*
        in_maps.append({
            "xT": np.ascontiguousarray(x[b].T),
            "WqT": np.ascontiguousarray(Wq[rs, :].T),
            "WkT": np.ascontiguousarray(Wk[rs, :].T),
            "WvT": np.ascontiguousarray(Wv[rs, :].T),
            "WlT": np.ascontiguousarray(Wlr[rs, :].T),
            "blr": np.ascontiguousarray(b_lr[rs][None, :]),
            "WoT": np.ascontiguousarray(Wo[:, rs].T),
        })
    res = bass_utils.run_bass_kernel_spmd(nc, in_maps, core_ids=list(range(8)))
    out = np.empty((B, T, D), np.float32)
    for b in range(B):
        out[b] = (res.results[2 * b]["y"].astype(np.float32)
                  + res.results[2 * b + 1]["y"].astype(np.float32))
    return out
